# revision 18
# baseline (speedup 1.0000x reference)
"""Crossformer (cross-attention + MLP) on 8 Trainium2 NeuronCores — v5.

v3 over v2:
  - ReduceScatter split per attention chunk (4 x [512,D] -> [64,D]) so the
    collectives pipeline behind attention instead of serializing at the end.
  - MLP block 0 emitted inside the last attention chunk (its mm1 fills the
    PE while ScalarE works through the final exp batch).
  - w2/b1 streamed mid-attention into the space freed by the x pool.
  - Single qT tile + 64-partition score matmuls (no zero-padded q copies).
  - ctx LN apply moved to ScalarE (scale/bias trick); DVE keeps stats only.
  - w1 DMA split into 2MB pieces placed late in the prologue / early
    attention so the first ctx tiles aren't queued behind it.
"""
import math
from contextlib import ExitStack

import numpy as np
import ml_dtypes

import concourse.bass as bass
import concourse.tile as tile
from concourse import bacc, mybir
from concourse.bass_utils import run_bass_kernel_spmd

F32 = mybir.dt.float32
BF16 = mybir.dt.bfloat16
I32 = mybir.dt.int32
FP8 = mybir.dt.float8e4
AF = mybir.ActivationFunctionType
ALU = mybir.AluOpType

NCORES = 8
N, D, C, H, HD, DFF = 2048, 1024, 768, 16, 64, 4096
EPS = 1e-12
HSD = D // NCORES
NCH = 512
NCHN = N // NCH            # 4 chunks == 4 RS pieces
RPC = N // NCORES          # 256 output rows per core
RPK = NCH // NCORES        # 64 rows per core per RS piece
RPB = 2 * RPK              # 128 rows per MLP block (2 RS pieces)
KOX = D // 128
KOC = C // 128
KOF = DFF // 128

_cache = {}


def build_program():
    nc = bacc.Bacc("TRN2", target_bir_lowering=False, debug=False,
                   num_devices=NCORES)

    def din(name, shape, dt=BF16):
        return nc.dram_tensor(name, shape, dt, kind="ExternalInput").ap()

    x_d = din("x", [N, D])
    ctx_d = din("ctx", [N, C])
    xrows_d = din("xrows", [RPC, D], F32)
    wq_d = din("wq", [D, HSD])
    wk_d = din("wk", [C, HSD])
    wv_d = din("wv", [C, HSD])
    wo0_d = din("wo0", [128, D])
    w1_d = din("w1", [128, KOX * DFF])
    w2_d = din("w2", [128, KOF * D])
    bq_d = din("bq", [HSD, 1], F32)
    bk_d = din("bk", [HSD, 1], F32)
    bv_d = din("bv", [HSD, 1], F32)
    b1_d = din("b1", [1, DFF])
    b2_d = din("b2", [1, D])
    id128_d = din("id128", [128, 128])
    id64s_d = din("id64s", [128, HD])
    out_d = nc.dram_tensor("out", [RPC, D], F32, kind="ExternalOutput").ap()

    with tile.TileContext(nc) as tc, ExitStack() as st:
        _build(nc, tc, st, locals())
    nc.compile()
    return nc


def _stats_tile(nc, pool, xt, d, mvp_t):
    fmax = math.gcd(512, d)
    nsub = d // fmax
    stt = pool.tile([128, nsub, 6], F32, tag="st", bufs=3)
    xg = xt.rearrange("p (s f) -> p s f", s=nsub)
    for s_ in range(nsub):
        nc.vector.bn_stats(out=stt[:, s_, :], in_=xg[:, s_, :])
    nc.vector.bn_aggr(out=mvp_t, in_=stt[:])


def _newton_rstd(nc, pool, mvp, nt):
    """DVE-only rsqrt(var+eps): bit-trick seed (fp32 arith) + 2 Newton steps."""
    vv = pool.tile([128, nt], F32, tag="nw_v", bufs=2)
    nc.vector.tensor_scalar(out=vv, in0=mvp[:, :, 1], scalar1=EPS,
                            scalar2=None, op0=ALU.add)
    fi = pool.tile([128, nt], F32, tag="nw_f", bufs=2)
    nc.vector.tensor_copy(out=fi, in_=vv[:, :].bitcast(I32))
    nc.vector.tensor_scalar(out=fi, in0=fi, scalar1=-0.5,
                            scalar2=1597463007.0, op0=ALU.mult, op1=ALU.add)
    yi = pool.tile([128, nt], I32, tag="nw_i", bufs=2)
    nc.vector.tensor_copy(out=yi, in_=fi)
    y = pool.tile([128, nt], F32, tag="nw_y", bufs=2)
    nc.vector.tensor_copy(out=y, in_=yi[:, :].bitcast(F32))
    tq = pool.tile([128, nt], F32, tag="nw_t", bufs=2)
    for _ in range(2):
        nc.vector.tensor_mul(out=tq, in0=y, in1=y)
        nc.vector.tensor_mul(out=tq, in0=tq, in1=vv)
        nc.vector.tensor_scalar(out=tq, in0=tq, scalar1=-0.5, scalar2=1.5,
                                op0=ALU.mult, op1=ALU.add)
        nc.vector.tensor_mul(out=y, in0=y, in1=tq)
    return y


def _apply_ln(nc, out_bf, xt, mean_ap, rstd_ap):
    nc.vector.tensor_scalar(out=out_bf, in0=xt, scalar1=mean_ap,
                            scalar2=rstd_ap, op0=ALU.subtract, op1=ALU.mult)


def _tp_group(nc, mmps, xss, j, dst, copy_eng, id128):
    tp = mmps.tile([128, 512], BF16, tag="mm")
    for t in range(4):
        nc.tensor.transpose(tp[:, t * 128:(t + 1) * 128],
                            xss[t][:, j * 128:(j + 1) * 128], id128)
    copy_eng(out=dst, in_=tp)


def _ctx_group(nc, P, g):
    cp, mmps = P["cp"], P["mmps"]
    cts = []
    for t in range(4):
        ct = cp.tile([128, C], BF16, tag="ct", bufs=5)
        nc.sync.dma_start(out=ct, in_=P["ctx_d"][(g * 4 + t) * 128:
                                                 (g * 4 + t + 1) * 128, :])
        cts.append(ct)
    mvp = cp.tile([128, 4, 2], F32, tag="cmvp", bufs=2)
    for t in range(4):
        _stats_tile(nc, cp, cts[t], C, mvp[:, t, :])
    rstd = _newton_rstd(nc, cp, mvp, 4)
    # bias for the ScalarE apply: -mean*rstd
    nmr = cp.tile([128, 4], F32, tag="cnm", bufs=2)
    nc.vector.tensor_mul(out=nmr, in0=mvp[:, :, 0], in1=rstd)
    nc.vector.tensor_scalar(out=nmr, in0=nmr, scalar1=-1.0, scalar2=None,
                            op0=ALU.mult)
    css = []
    for t in range(4):
        cs = cp.tile([128, C], BF16, tag="cs", bufs=5)
        nc.scalar.activation(out=cs, in_=cts[t], func=AF.Identity,
                             bias=nmr[:, t:t + 1], scale=rstd[:, t:t + 1])
        css.append(cs)
    for j in range(KOC):
        eng = nc.scalar.copy if j % 2 == 0 else nc.vector.tensor_copy
        _tp_group(nc, mmps, css, j, P["csT"][:, j, g * 512:(g + 1) * 512],
                  eng, P["id128"])
    sl = slice(g * NCH, (g + 1) * NCH)
    for w_sb, bias, dstT in ((P["wk_sb"], P["bk_sb"], P["kT"]),
                             (P["wv_sb"], P["bv_sb"], P["vT"])):
        ps = mmps.tile([128, NCH], F32, tag="mm")
        for k in range(KOC):
            nc.tensor.matmul(ps, lhsT=w_sb[:, k, :], rhs=P["csT"][:, k, sl],
                             start=(k == 0), stop=(k == KOC - 1))
        nc.scalar.activation(out=dstT[:, sl], in_=ps, func=AF.Identity,
                             bias=bias, scale=1.0)
    for h in range(2):
        tp = mmps.tile([128, 512], BF16, tag="mm")
        for t in range(4):
            mo = g * 4 + t
            nc.tensor.transpose(
                tp[:, t * 128:t * 128 + HD],
                P["vT"][h * HD:(h + 1) * HD, mo * 128:(mo + 1) * 128],
                P["id64s"][h * HD:(h + 1) * HD, :])
        nc.vector.tensor_copy(
            out=P["vN"][:, g * 4:(g + 1) * 4, h * (HD + 1):h * (HD + 1) + HD],
            in_=tp.rearrange("p (t f) -> p t f", t=4)[:, :, :HD])


def _prep_x(nc, P, g):
    xp, mmps = P["xp"], P["mmps"]
    xts = []
    for t in range(4):
        xt = xp.tile([128, D], BF16, tag="xt", bufs=5)
        nc.sync.dma_start(out=xt, in_=P["x_d"][(g * 4 + t) * 128:
                                               (g * 4 + t + 1) * 128, :])
        xts.append(xt)
    mvp = xp.tile([128, 4, 2], F32, tag="mvp", bufs=2)
    for t in range(4):
        _stats_tile(nc, xp, xts[t], D, mvp[:, t, :])
    rstd = _newton_rstd(nc, xp, mvp, 4)
    xss = []
    for t in range(4):
        xs = xp.tile([128, D], BF16, tag="xs", bufs=5)
        _apply_ln(nc, xs, xts[t], mvp[:, t, 0:1], rstd[:, t:t + 1])
        xss.append(xs)
    xsT = xp.tile([128, KOX, NCH], BF16, tag="xsT", bufs=2)
    for j in range(KOX):
        _tp_group(nc, mmps, xss, j, xsT[:, j, :], nc.vector.tensor_copy,
                  P["id128"])
    sl = slice(g * NCH, (g + 1) * NCH)
    ps = mmps.tile([128, NCH], F32, tag="mm")
    for k in range(KOX):
        nc.tensor.matmul(ps, lhsT=P["wq_sb"][:, k, :], rhs=xsT[:, k, :],
                         start=(k == 0), stop=(k == KOX - 1))
    nc.scalar.activation(out=P["qT"][:, sl], in_=ps, func=AF.Identity,
                         bias=P["bq_sb"], scale=1.0)


def _scores(nc, P, ch):
    scps, ptp = P["scps"], P["ptp"]
    sl = slice(ch * NCH, (ch + 1) * NCH)
    pts = [ptp.tile([128, N // 128, NCH], BF16, tag="pt", bufs=2,
                    name=f"pt{ch}_{h}") for h in range(2)]
    for mop in range(N // 256):
        sc0 = scps.tile([128, 2 * NCH], F32, tag="sc")
        sc1 = scps.tile([128, 2 * NCH], F32, tag="sc")
        for t in range(2):
            mo = 2 * mop + t
            for h, scx in ((0, sc0), (1, sc1)):
                hsl = slice(h * HD, (h + 1) * HD)
                nc.tensor.matmul(
                    scx[:, t * NCH:(t + 1) * NCH],
                    lhsT=P["kT"][hsl, mo * 128:(mo + 1) * 128],
                    rhs=P["qT"][hsl, sl], start=True, stop=True,
                    tile_position=(h * HD, 0))
        for h, scx in ((0, sc0), (1, sc1)):
            # exp(s + ln16): fp8 numerators stay in normal range; the
            # x16 cancels exactly in the rsum normalization
            dst = pts[h][:, 2 * mop:2 * mop + 2, :].rearrange(
                "p a b -> p (a b)")
            nc.scalar.activation(out=dst, in_=scx, func=AF.Exp,
                                 bias=P["lnp_sb"], scale=1.0)
    return pts


def _attn_av(nc, P, ch, pts):
    """P@V + per-token normalization into oTs for chunk ch."""
    mmps, att = P["mmps"], P["att"]
    sl = slice(ch * NCH, (ch + 1) * NCH)
    for h in range(2):
        pt = pts[h]
        po = mmps.tile([HD + 1, NCH], F32, tag="mm")
        for mo in range(N // 128):
            nc.tensor.matmul(
                po, lhsT=P["vN"][:, mo, h * (HD + 1):(h + 1) * (HD + 1)],
                rhs=pt[:, mo, :], start=(mo == 0), stop=(mo == N // 128 - 1))
        rsum = att.tile([1, NCH], F32, tag="rec")
        nc.vector.tensor_copy(out=rsum, in_=po[HD:HD + 1, :])
        rb_ps = mmps.tile([HD, NCH], F32, tag="mm")
        nc.tensor.matmul(rb_ps, lhsT=P["ones1f"][:, :HD], rhs=rsum,
                         start=True, stop=True)
        rb = att.tile([HD, NCH], F32, tag="rb_sb")
        nc.vector.reciprocal_approx_fast(out=rb, in_=rb_ps)
        if h == 0:
            nc.vector.tensor_mul(out=P["oTs"][:HD, sl], in0=po[:HD, :],
                                 in1=rb)
        else:
            nc.vector.tensor_mul(out=P["oT1"][:, sl], in0=po[:HD, :], in1=rb)
            nc.sync.dma_start(out=P["oTs"][HD:, sl], in_=P["oT1"][:, sl])


def _attn_op(nc, P, ch):
    """Output projection of chunk ch into its own RS piece, then trigger RS."""
    mmps, opb = P["mmps"], P["opb"]
    for nt in range(NCH // 128):
        osl = slice(ch * NCH + nt * 128, ch * NCH + (nt + 1) * 128)
        op_t = opb.tile([128, D], BF16, tag="op")
        for c2 in range(D // 512):
            pp = mmps.tile([128, 512], F32, tag="mm")
            nc.tensor.matmul(pp, lhsT=P["oTs"][:, osl],
                             rhs=P["wo_sb"][:, c2 * 512:(c2 + 1) * 512],
                             start=True, stop=True)
            if c2 == 0:
                nc.vector.tensor_copy(out=op_t[:, c2 * 512:(c2 + 1) * 512],
                                      in_=pp)
            else:
                nc.scalar.copy(out=op_t[:, c2 * 512:(c2 + 1) * 512], in_=pp)
        nc.sync.dma_start(
            out=P["op_s"][ch][nt * 128:(nt + 1) * 128, :], in_=op_t)
    with nc.named_scope(f"rs{ch}"):
        nc.gpsimd.collective_compute(
            "ReduceScatter", ALU.add,
            replica_groups=[list(range(NCORES))],
            ins=[P["op_s"][ch].opt()], outs=[P["rs_s"][ch].opt()])


def _mlp_front(nc, P, s):
    """Residual + LN + transpose for MLP block s (rows from RS pieces 2s,2s+1)."""
    mb, mmps = P["mb"], P["mmps"]
    rs_bf = mb.tile([128, D], BF16, tag="rsb", bufs=2)
    nc.sync.dma_start(out=rs_bf[:RPK, :], in_=P["rs_s"][2 * s])
    nc.sync.dma_start(out=rs_bf[RPK:, :], in_=P["rs_s"][2 * s + 1])
    xr = mb.tile([128, D], F32, tag="xr", bufs=2)
    nc.sync.dma_start(out=xr, in_=P["xrows_d"][s * RPB:(s + 1) * RPB, :])
    xnew = mb.tile([128, D], F32, tag="xnew", bufs=2)
    nc.vector.tensor_add(out=xnew, in0=xr, in1=rs_bf)
    mvp = mb.tile([128, 1, 2], F32, tag="mmvp", bufs=2)
    _stats_tile(nc, mb, xnew, D, mvp[:, 0, :])
    rstd = _newton_rstd(nc, mb, mvp, 1)
    xms = mb.tile([128, D], BF16, tag="xms", bufs=2)
    _apply_ln(nc, xms, xnew, mvp[:, 0, 0:1], rstd[:, 0:1])
    xmsT = mb.tile([128, KOX, 128], BF16, tag="xmsT", bufs=2)
    for jg in range(KOX // 4):
        tp = mmps.tile([128, 512], BF16, tag="mm")
        for t in range(4):
            j = jg * 4 + t
            nc.tensor.transpose(tp[:, t * 128:(t + 1) * 128],
                                xms[:, j * 128:(j + 1) * 128], P["id128"])
        nc.vector.tensor_copy(out=xmsT[:, jg * 4:(jg + 1) * 4, :],
                              in_=tp.rearrange("p (t f) -> p t f", t=4))
    return xnew, xmsT


def _mlp_mm1_half(nc, P, xmsT, half):
    scps = P["scps"]
    h0 = half * 2048
    pga = scps.tile([128, 1024], F32, tag="sc")
    pgb = scps.tile([128, 1024], F32, tag="sc")
    regs = [pga[:, :512], pga[:, 512:], pgb[:, :512], pgb[:, 512:]]
    for k in range(KOX):
        for r in range(4):
            nc.tensor.matmul(
                regs[r], lhsT=xmsT[:, k, :],
                rhs=P["w1_sb"][:, k, h0 + r * 512:h0 + (r + 1) * 512],
                start=(k == 0), stop=False)
    for r in range(4):
        nc.tensor.matmul(
            regs[r], lhsT=P["ones1"][:, :128],
            rhs=P["b1_sb"][:, h0 + r * 512:h0 + (r + 1) * 512],
            start=False, stop=True)
    return pga, pgb


def _mlp_rest(nc, P, s, xnew, xmsT, pre=None):
    """gelu + gT + mm2 (+ b2) + residual + store for block s.

    pre: optionally the already-emitted (pga, pgb) for half 0."""
    mb, mmps = P["mb"], P["mmps"]
    g_sb = mb.tile([128, DFF], BF16, tag="g", bufs=2)
    gT = mb.tile([128, KOF, 128], BF16, tag="gT", bufs=2)
    p2s = [mmps.tile([128, 512], F32, tag="mm", name=f"p2_{s}_{c}")
           for c in range(2)]
    for half in range(2):
        h0 = half * 2048
        if half == 0 and pre is not None:
            pga, pgb = pre
        else:
            pga, pgb = _mlp_mm1_half(nc, P, xmsT, half)
        nc.scalar.activation(out=g_sb[:, h0:h0 + 1024], in_=pga,
                             func=AF.Gelu_apprx_tanh)
        nc.scalar.activation(out=g_sb[:, h0 + 1024:h0 + 2048], in_=pgb,
                             func=AF.Gelu_apprx_tanh)
        for jg in range(4):
            tp = mmps.tile([128, 512], BF16, tag="mm")
            for t in range(4):
                j = half * 16 + jg * 4 + t
                nc.tensor.transpose(tp[:, t * 128:(t + 1) * 128],
                                    g_sb[:, j * 128:(j + 1) * 128],
                                    P["id128"])
            nc.vector.tensor_copy(
                out=gT[:, half * 16 + jg * 4:half * 16 + (jg + 1) * 4, :],
                in_=tp.rearrange("p (t f) -> p t f", t=4))
        for q in range(2):
            w2q = P["w2h"][half * 2 + q]
            for k in range(8):
                kk = half * 16 + q * 8 + k
                nc.tensor.matmul(p2s[0], lhsT=gT[:, kk, :],
                                 rhs=w2q[:, k, 0:512],
                                 start=(kk == 0), stop=False)
                nc.tensor.matmul(p2s[1], lhsT=gT[:, kk, :],
                                 rhs=w2q[:, k, 512:1024],
                                 start=(kk == 0), stop=False)
    for c in range(2):
        nc.tensor.matmul(p2s[c], lhsT=P["ones1"][:, :128],
                         rhs=P["b2_sb"][:, c * 512:(c + 1) * 512],
                         start=False, stop=True)
    out_sb = mb.tile([128, D], F32, tag="osb", bufs=1)
    for c in range(2):
        csl = slice(c * 512, (c + 1) * 512)
        nc.vector.tensor_add(out=out_sb[:, csl], in0=p2s[c],
                             in1=xnew[:, csl])
    nc.sync.dma_start(out=P["out_d"][s * RPB:(s + 1) * RPB, :], in_=out_sb)


def _build(nc, tc, st, d):
    P = {k: d[k] for k in ("x_d", "ctx_d", "xrows_d", "out_d")}

    const = st.enter_context(tc.tile_pool(name="const", bufs=1))
    dram = st.enter_context(tc.tile_pool(name="dram", bufs=1, space="DRAM"))
    persist = st.enter_context(tc.tile_pool(name="persist", bufs=1))
    P["mmps"] = st.enter_context(tc.tile_pool(name="mmps", bufs=4,
                                              space="PSUM"))
    P["scps"] = st.enter_context(tc.tile_pool(name="scps", bufs=2,
                                              space="PSUM"))

    # ---- constants ----
    id128 = const.tile([128, 128], BF16)
    nc.sync.dma_start(out=id128, in_=d["id128_d"])
    id64s = const.tile([128, HD], BF16)
    nc.sync.dma_start(out=id64s, in_=d["id64s_d"])
    eps_sb = const.tile([128, 1], F32)
    nc.vector.memset(eps_sb, EPS)
    lnp_sb = const.tile([128, 1], F32)
    nc.vector.memset(lnp_sb, float(math.log(16.0)))
    ones1 = const.tile([1, 128], BF16)
    nc.vector.memset(ones1, 1.0)
    ones1f = const.tile([1, 128], F32)
    nc.vector.memset(ones1f, 1.0)
    bq_sb = const.tile([HSD, 1], F32)
    nc.sync.dma_start(out=bq_sb, in_=d["bq_d"])
    bk_sb = const.tile([HSD, 1], F32)
    nc.sync.dma_start(out=bk_sb, in_=d["bk_d"])
    bv_sb = const.tile([HSD, 1], F32)
    nc.sync.dma_start(out=bv_sb, in_=d["bv_d"])
    b2_sb = const.tile([1, D], BF16)
    nc.sync.dma_start(out=b2_sb, in_=d["b2_d"])
    b1_sb = const.tile([1, DFF], BF16)
    nc.sync.dma_start(out=b1_sb, in_=d["b1_d"])
    P["b1_sb"] = b1_sb
    wo_sb = const.tile([128, D], BF16)
    nc.sync.dma_start(out=wo_sb, in_=d["wo0_d"])
    P.update(id128=id128, id64s=id64s, eps_sb=eps_sb, lnp_sb=lnp_sb,
             ones1=ones1, ones1f=ones1f, bq_sb=bq_sb, bk_sb=bk_sb,
             bv_sb=bv_sb, b2_sb=b2_sb, wo_sb=wo_sb)

    w1_sb = persist.tile([128, KOX, DFF], BF16)
    P["w1_sb"] = w1_sb
    # first w2 quarter is persistent so its DMA can run during attention
    w2q0 = persist.tile([128, KOF // 4, D], BF16)
    w1_src = d["w1_d"].rearrange("p (ko m) -> p ko m", ko=KOX)
    w2_src = d["w2_d"].rearrange("p (ko m) -> p ko m", ko=KOF)

    op_s = [dram.tile([NCH, D], BF16, name=f"op{i}") for i in range(NCHN)]
    rs_s = [dram.tile([RPK, D], BF16, name=f"rs{i}") for i in range(NCHN)]
    P.update(op_s=op_s, rs_s=rs_s)

    # ---- pools (LIFO: aper > xp > cp/ptp-era) ----
    ap_cm = tc.tile_pool(name="aper", bufs=1)
    aper = ap_cm.__enter__()
    xp_cm = tc.tile_pool(name="xp", bufs=1)
    xp = xp_cm.__enter__()
    P["xp"] = xp

    qT = aper.tile([128, N], BF16)
    kT = aper.tile([128, N], BF16)
    vN = aper.tile([128, N // 128, 2 * (HD + 1)], BF16)
    oTs = aper.tile([128, N], BF16)
    oT1 = aper.tile([HD, N], BF16)
    nc.vector.memset(vN[:, :, HD:HD + 1], 1.0)
    nc.vector.memset(vN[:, :, 2 * HD + 1:], 1.0)
    P.update(qT=qT, kT=kT, vN=vN, oTs=oTs, oT1=oT1)

    cp_cm = tc.tile_pool(name="cp", bufs=1)
    cp = cp_cm.__enter__()
    P["cp"] = cp
    csT = cp.tile([128, KOC, N], BF16)
    vT = cp.tile([128, N], BF16)
    wk_sb = cp.tile([128, KOC, HSD], BF16)
    wv_sb = cp.tile([128, KOC, HSD], BF16)
    P.update(csT=csT, vT=vT, wk_sb=wk_sb, wv_sb=wv_sb)
    wq_sb = xp.tile([128, KOX, HSD], BF16)
    P["wq_sb"] = wq_sb

    with nc.named_scope("pro"):
        # small weights first, then the first ctx tiles — w1 comes later
        nc.sync.dma_start(out=wk_sb, in_=d["wk_d"].rearrange(
            "(ko ki) m -> ki ko m", ki=128))
        nc.sync.dma_start(out=wv_sb, in_=d["wv_d"].rearrange(
            "(ko ki) m -> ki ko m", ki=128))
        for g in range(NCHN):
            _ctx_group(nc, P, g)
            if g == 0:
                nc.sync.dma_start(out=wq_sb, in_=d["wq_d"].rearrange(
                    "(ko ki) m -> ki ko m", ki=128))
            if g >= 2:
                p_ = g - 2
                nc.sync.dma_start(
                    out=w1_sb[:, p_ * 2:(p_ + 1) * 2, :],
                    in_=w1_src[:, p_ * 2:(p_ + 1) * 2, :])
        _prep_x(nc, P, 0)
    cp_cm.__exit__(None, None, None)
    nc.sync.dma_start(out=w2q0, in_=w2_src[:, 0:KOF // 4, :])

    ptp_cm = tc.tile_pool(name="ptp", bufs=2)
    P["ptp"] = ptp_cm.__enter__()
    att_cm = tc.tile_pool(name="att", bufs=2)
    P["att"] = att_cm.__enter__()
    opb_cm = tc.tile_pool(name="opb", bufs=2)
    P["opb"] = opb_cm.__enter__()

    with nc.named_scope("attn"):
        for ch in range(NCHN):
            pts = _scores(nc, P, ch)
            if ch < 2:
                p_ = ch + 2
                nc.sync.dma_start(
                    out=w1_sb[:, p_ * 2:(p_ + 1) * 2, :],
                    in_=w1_src[:, p_ * 2:(p_ + 1) * 2, :])
            if ch + 1 < NCHN:
                _prep_x(nc, P, ch + 1)
            _attn_av(nc, P, ch, pts)
            _attn_op(nc, P, ch)

    opb_cm.__exit__(None, None, None)
    att_cm.__exit__(None, None, None)
    ptp_cm.__exit__(None, None, None)
    xp_cm.__exit__(None, None, None)
    ap_cm.__exit__(None, None, None)

    wp_cm = tc.tile_pool(name="wp2", bufs=1)
    wp = wp_cm.__enter__()
    mb_cm = tc.tile_pool(name="mb", bufs=1)
    P["mb"] = mb_cm.__enter__()
    P["w2h"] = [w2q0] + [wp.tile([128, KOF // 4, D], BF16, name=f"w2q{i}")
                         for i in range(1, 4)]
    with nc.named_scope("mlp"):
        xnew0, xmsT0 = _mlp_front(nc, P, 0)
        for i in range(1, 4):
            nc.sync.dma_start(
                out=P["w2h"][i],
                in_=w2_src[:, i * (KOF // 4):(i + 1) * (KOF // 4), :])
        _mlp_rest(nc, P, 0, xnew0, xmsT0)
        xnew1, xmsT1 = _mlp_front(nc, P, 1)
        _mlp_rest(nc, P, 1, xnew1, xmsT1)
    mb_cm.__exit__(None, None, None)
    wp_cm.__exit__(None, None, None)


def shard_inputs(inputs):
    bf = ml_dtypes.bfloat16
    x = np.asarray(inputs["x"], np.float32)
    ctx = np.asarray(inputs["ctx"], np.float32)
    qn_w, qn_b = np.asarray(inputs["qn_w"], np.float32), np.asarray(inputs["qn_b"], np.float32)
    kvn_w, kvn_b = np.asarray(inputs["kvn_w"], np.float32), np.asarray(inputs["kvn_b"], np.float32)
    pn_w, pn_b = np.asarray(inputs["pn_w"], np.float32), np.asarray(inputs["pn_b"], np.float32)
    wq, bq = np.asarray(inputs["wq"], np.float32), np.asarray(inputs["bq"], np.float32)
    wk, bk = np.asarray(inputs["wk"], np.float32), np.asarray(inputs["bk"], np.float32)
    wv, bv = np.asarray(inputs["wv"], np.float32), np.asarray(inputs["bv"], np.float32)
    wo, bo = np.asarray(inputs["wo"], np.float32), np.asarray(inputs["bo"], np.float32)
    w1, b1 = np.asarray(inputs["w1"], np.float32), np.asarray(inputs["b1"], np.float32)
    w2, b2 = np.asarray(inputs["w2"], np.float32), np.asarray(inputs["b2"], np.float32)

    s = 1.0 / math.sqrt(HD)
    wq_f = (qn_w[:, None] * wq) * s
    bq_f = (qn_b @ wq + bq) * s
    wk_f = kvn_w[:, None] * wk
    bk_f = kvn_b @ wk + bk
    wv_f = kvn_w[:, None] * wv
    bv_f = kvn_b @ wv + bv
    w1_f = (pn_w[:, None] * w1).astype(bf)
    w1_f = np.ascontiguousarray(
        w1_f.reshape(KOX, 128, DFF).transpose(1, 0, 2).reshape(128, KOX * DFF))
    b1_f = (pn_b @ w1 + b1).astype(bf).reshape(1, DFF)
    w2_b = np.ascontiguousarray(
        w2.astype(bf).reshape(KOF, 128, D).transpose(1, 0, 2).reshape(128, KOF * D))
    id128 = np.eye(128, dtype=bf)
    id64s = np.concatenate([np.eye(HD), np.eye(HD)]).astype(bf)
    x_bf = x.astype(bf)
    ctx_bf = ctx.astype(bf)

    in_maps = []
    for c in range(NCORES):
        hs = slice(c * HSD, (c + 1) * HSD)
        xrows = np.concatenate(
            [x[i * NCH + c * RPK: i * NCH + (c + 1) * RPK]
             for i in range(NCHN)]) + bo[None, :]
        woc = wo[hs, :]
        in_maps.append({
            "x": x_bf, "ctx": ctx_bf,
            "xrows": np.ascontiguousarray(xrows).astype(np.float32),
            "wq": wq_f[:, hs].astype(bf), "wk": wk_f[:, hs].astype(bf),
            "wv": wv_f[:, hs].astype(bf),
            "wo0": np.ascontiguousarray(woc).astype(bf),
            "w1": w1_f, "w2": w2_b,
            "bq": bq_f[hs].reshape(-1, 1).astype(np.float32),
            "bk": bk_f[hs].reshape(-1, 1).astype(np.float32),
            "bv": bv_f[hs].reshape(-1, 1).astype(np.float32),
            "b1": b1_f, "b2": b2.reshape(1, -1).astype(bf),
            "id128": id128, "id64s": id64s,
        })
    return in_maps


def gather_output(results):
    out = np.empty((N, D), np.float32)
    for c in range(NCORES):
        r = results[c]["out"]
        for i in range(NCHN):
            out[i * NCH + c * RPK: i * NCH + (c + 1) * RPK] = \
                r[i * RPK:(i + 1) * RPK]
    return out


def run(inputs, trace=False, **kw):
    if "nc" not in _cache:
        _cache["nc"] = build_program()
    nc = _cache["nc"]
    in_maps = shard_inputs(inputs)
    res = run_bass_kernel_spmd(nc, in_maps, core_ids=list(range(NCORES)),
                               trace=trace, **kw)
    return gather_output(res.results), res


def kernel(**inputs):
    out, _ = run(inputs, trace=False)
    return out

